# revision 83
# baseline (speedup 1.0000x reference)
"""Multi-head attention (B=8, S=1024, D=1024, H=16) on 8 TRN2 NeuronCores.

Sharding: pure data parallel - batch element b on core b. Weights broadcast.

Single-core schedule: a 16-window (head-pair x seq-half) software pipeline
keeping ScalarE (exp, ~147us) and TensorE (~200us of matmul streaming)
concurrent from ~15us:

  head:    X^T via PE transposes (ScalarE drains them so DVE stays free
           for casts); Q/K projection for pair 0; window-0 scores fire as
           soon as s-tiles 0-3 land, starting the exp spine at ~20us.
  windows: per (pair, sc) window the 8 score matmuls + one wide exp per
           k-tile set the ACT pace; PV steps for the trailing window and
           filler (remaining projections, V projection, W_out staging)
           absorb TensorE idle. Hard deadlines force projection emission
           before the window that reads it.
  PV:      ones-augmented V (M=65) accumulates unnormalized out^T plus the
           softmax denominator; denominators are reshaped [1,512]->[4,128]
           by SBUF-SBUF DMA so one [8,128] DVE reciprocal per window
           replaces 2 single-partition 3.3us reciprocals.
  tail:    remaining PV windows + normalize, then Y = attn_out @ W_out.
"""

import sys

sys.path.insert(0, "/opt/trn_rl_repo")

import numpy as np

import concourse.bacc as bacc
import concourse.mybir as mybir
from concourse.bass_utils import run_bass_kernel_spmd
from concourse.masks import make_identity
from concourse.tile import TileContext

B = 8
S = 1024
D = 1024
H = 16
DK = D // H  # 64
P = 128
ST = S // P   # 8 s-tiles
DT = D // P   # 8 d-tiles
NTQK = 2 * D // P  # 16 n-tiles for the Q|K part
PAIRS = H // 2     # 8 head pairs
NW = 2 * PAIRS     # 16 windows (pair, sc)

f32 = mybir.dt.float32
bf16 = mybir.dt.bfloat16
EXP = mybir.ActivationFunctionType.Exp
MULT = mybir.AluOpType.mult
ADD = mybir.AluOpType.add


def build_nc():
    nc = bacc.Bacc()
    X = nc.dram_tensor("X", [S, D], f32, kind="ExternalInput")
    W_in = nc.dram_tensor("W_in", [D, 3 * D], f32, kind="ExternalInput")
    b_in = nc.dram_tensor("b_in", [3 * D], f32, kind="ExternalInput")
    W_out = nc.dram_tensor("W_out", [D, D], f32, kind="ExternalInput")
    b_out = nc.dram_tensor("b_out", [D], f32, kind="ExternalInput")
    out = nc.dram_tensor("out", [S, D], f32, kind="ExternalOutput")

    w_in_kp = W_in.rearrange("(ko p) n -> p ko n", p=P)  # [128, 8, 3072]
    w_out_kp = W_out.rearrange("(ko p) n -> p ko n", p=P)  # [128, 8, 1024]

    with TileContext(nc) as tc:
        # ------------------------------------------------ constants
        const = tc.alloc_tile_pool(name="const", bufs=1)
        bqk = const.tile([P, NTQK], f32)
        bv_bc = const.tile([P, D], f32)
        bout_bc = const.tile([P, D], f32)
        ones4 = const.tile([P, ST, H, 1], f32)
        nc.vector.memset(ones4[:], 1.0)

        # ------------------------------------------------ resident (left)
        qkT_pool = tc.alloc_tile_pool(name="qkT", bufs=1)
        qkT = qkT_pool.tile([P, NTQK, S], bf16)  # 32 KB/p
        vaug_pool = tc.alloc_tile_pool(name="vaug", bufs=1)
        v_aug = vaug_pool.tile([P, ST, H, DK + 1], bf16)  # 16.6 KB/p
        nc.vector.tensor_copy(v_aug[:, :, :, DK : DK + 1], ones4[:])
        attnT_pool = tc.alloc_tile_pool(name="attnT", bufs=1)
        attnT = attnT_pool.tile([P, PAIRS, S], bf16)  # 16 KB/p

        pvs_pool = tc.alloc_tile_pool(name="pvs", bufs=3)   # 6 KB/p
        dnc_pool = tc.alloc_tile_pool(name="dnc", bufs=2)
        rc_pool = tc.alloc_tile_pool(name="rcp", bufs=2)
        rr_pool = tc.alloc_tile_pool(name="rrp", bufs=2)
        bc_pool = tc.alloc_tile_pool(name="bcp", bufs=2)

        # ------------------------------------------------ transient (right)
        xT_pool = tc.alloc_tile_pool(name="xTp", bufs=1, side="right")
        xT = xT_pool.tile([P, DT, S], bf16)  # 16 KB/p
        wqk_stage = tc.alloc_tile_pool(name="wqks", bufs=2, side="right")
        wqk_bfp = tc.alloc_tile_pool(name="wqkb", bufs=2, side="right")
        wv_pool = tc.alloc_tile_pool(name="wvb", bufs=1, side="right")
        wv_bf = wv_pool.tile([P, DT, D], bf16)  # 16 KB/p
        ex_pool = tc.alloc_tile_pool(name="exp", bufs=22, side="right")  # 44
        wv_stage = tc.alloc_tile_pool(name="wvs", bufs=1, side="right")

        def emit_wqk_stage(nt):
            ws = wqk_stage.tile([P, DT, P], f32, tag="ws", name="ws")
            nc.sync.dma_start(ws[:], w_in_kp[:, :, nt * P : (nt + 1) * P])
            wb = wqk_bfp.tile([P, DT, P], bf16, tag="wb", name="wb")
            nc.vector.tensor_copy(wb[:], ws[:])
            return wb

        wb_refs = {}

        # ------------------------------------------------ PSUM pools
        # bce (filler/E) 2 banks, sps 4 banks, pv 2 banks = 8
        bce_ps = tc.alloc_tile_pool(name="bceps", bufs=2, space="PSUM")
        sps_pool = tc.alloc_tile_pool(name="spsp", bufs=2, space="PSUM")
        pv_ps = tc.alloc_tile_pool(name="pvps", bufs=2, space="PSUM")

        # phase A staging
        xstage = tc.alloc_tile_pool(name="xstage", bufs=2, side="right")
        xbp = tc.alloc_tile_pool(name="xbp", bufs=2, side="right")
        rowp = tc.alloc_tile_pool(name="rowp", bufs=1, side="right")

        identity = const.tile([P, P], bf16)
        make_identity(nc, identity[:])

        def emit_a_step(si, eng=None):
            x_tile = xstage.tile([P, D], f32, tag="x", name="xt")
            (eng or nc.sync).dma_start(x_tile[:], X[si * P : (si + 1) * P, :])
            xb = xbp.tile([P, D], bf16, tag="xb", name="xb")
            nc.vector.tensor_copy(xb[:], x_tile[:])
            for half in range(2):
                tp4 = bce_ps.tile([P, 4, P], f32, tag="bce", name="tp4")
                for dj in range(4):
                    dj_abs = half * 4 + dj
                    nc.tensor.matmul(
                        tp4[:, dj, :],
                        xb[:, dj_abs * P : (dj_abs + 1) * P],
                        identity[:],
                        start=True,
                        stop=True,
                    )
                # ScalarE is idle during the head; keep DVE free
                nc.scalar.copy(
                    xT[:, half * 4 : (half + 1) * 4, si * P : (si + 1) * P],
                    tp4[:],
                )

        for si in range(4):
            # odd tiles load via the gpsimd DMA queue: two queues in
            # flight halve the head's X staging latency
            if si == 3:
                # HAM warmup: one contiguous ~3.9us junk-matmul burst
                # (covering the si3 DMA wait) un-throttles the PE clock so
                # si3's transposes and pair-0's projections run at 2.4GHz
                warm = bce_ps.tile([P, P], f32, tag="bce", name="warm")
                for _ in range(36):
                    nc.tensor.matmul(warm[:], identity[:], identity[:],
                                     start=True, stop=True)
            emit_a_step(si, eng=nc.gpsimd if si % 2 else nc.sync)
            if si == 1:
                wb_refs[0] = {"wb": emit_wqk_stage(0)}
            elif si == 2:
                wb_refs[8] = {"wb": emit_wqk_stage(8)}
            elif si == 3:
                brow = rowp.tile([1, D], f32, tag="r", name="bvrow")
                nc.sync.dma_start(brow[:], b_in[None, 2 * D : 3 * D])
                nc.gpsimd.partition_broadcast(bv_bc[:], brow[:])
                nc.sync.dma_start(
                    bqk[:], b_in[0 : 2 * D].rearrange("(o p) -> p o", p=P)
                )

        # ------------------------------------------------ helpers
        def emit_wv_chunk(ci):
            # chunk ci: dk pair (2*ci, 2*ci+1), all 1024 V columns
            wvs = wv_stage.tile([P, 2, D], f32, tag="wvs", name="wvs")
            nc.sync.dma_start(
                wvs[:], w_in_kp[:, 2 * ci : 2 * ci + 2, 2 * D : 3 * D]
            )
            nc.vector.tensor_copy(wv_bf[:, 2 * ci : 2 * ci + 2, :], wvs[:])

        def make_b_group(nt, sc, wb_ref):
            ps_ref = {}

            def mm(dk):
                if dk == 0:
                    ps_ref["ps"] = bce_ps.tile([P, 512], f32, tag="bce", name="psb")
                nc.tensor.matmul(
                    ps_ref["ps"][:],
                    wb_ref["wb"][:, dk, :],
                    xT[:, dk, sc * 512 : (sc + 1) * 512],
                    start=(dk == 0),
                    stop=(dk == DT - 1),
                )

            def drain():
                nc.vector.tensor_scalar_add(
                    qkT[:, nt, sc * 512 : (sc + 1) * 512],
                    ps_ref["ps"][:],
                    bqk[:, nt : nt + 1],
                )

            return {"mms": [lambda dk=dk: mm(dk) for dk in range(DT)],
                    "drain": drain}

        def make_c_group(st, ncx):
            ps_ref = {}

            def mm(dk):
                if dk == 0:
                    ps_ref["ps"] = bce_ps.tile([P, 512], f32, tag="bce", name="psc")
                nc.tensor.matmul(
                    ps_ref["ps"][:],
                    xT[:, dk, st * P : (st + 1) * P],
                    wv_bf[:, dk, ncx * 512 : (ncx + 1) * 512],
                    start=(dk == 0),
                    stop=(dk == DT - 1),
                )

            def drain():
                nc.vector.tensor_tensor(
                    v_aug[:, st, ncx * 8 : (ncx + 1) * 8, 0:DK],
                    ps_ref["ps"][:].rearrange("p (h d) -> p h d", d=DK),
                    bv_bc[:, ncx * 512 : (ncx + 1) * 512].rearrange(
                        "p (h d) -> p h d", d=DK
                    ),
                    ADD,
                )
                c_emitted[ncx] += 1

            return {"mms": [lambda dk=dk: mm(dk) for dk in range(DT)],
                    "drain": drain}

        c_emitted = {0: 0, 1: 0}

        def c_done(ncx):
            return c_emitted[ncx] >= ST

        # ---- filler worklist (deadline ordered) ------------------------
        filler_plan = []

        def plan_b(nt):
            filler_plan.append(("stage_b", nt))
            filler_plan.append(("bgroup", nt, 0))
            filler_plan.append(("bgroup", nt, 1))

        # weave: each nt's weight stage (DMA+cast) lands one group-pair
        # ahead of its matmuls so the cast never blocks the PE
        b_order = [1, 9, 2, 10, 3, 11, 4, 12, 5, 13, 6, 14, 7, 15]
        other = (
            [("stage_wv", ci) for ci in range(4)]
            + [("cgroup", st, 0) for st in range(ST)]
            + [("cgroup", st, 1) for st in range(ST)]
            + [("stage_wo", ko) for ko in range(DT)]
        )
        oi = [0]

        def take_other(n):
            got = other[oi[0] : oi[0] + n]
            oi[0] += len(got)
            return got

        filler_plan.append(("stage_b", b_order[0]))
        filler_plan.append(("stage_b", b_order[1]))
        filler_plan.extend(take_other(2))  # wv chunks 0,1 early
        for k, nt in enumerate(b_order):
            filler_plan.append(("bgroup", nt, 0))
            if k + 2 < len(b_order):
                filler_plan.append(("stage_b", b_order[k + 2]))
            filler_plan.append(("bgroup", nt, 1))
            filler_plan.extend(take_other(2))
        filler_plan.extend(other[oi[0] :])

        fstate = {"i": 0, "group": None, "mmi": 0, "item": None}
        b_done = {0: 0, 8: 0}

        def filler_exhausted():
            return fstate["group"] is None and fstate["i"] >= len(filler_plan)

        def emit_filler_unit(max_mms=2):
            cyc = 0
            mms = 0
            while mms < max_mms:
                if fstate["group"] is None:
                    if fstate["i"] >= len(filler_plan):
                        return cyc
                    item = filler_plan[fstate["i"]]
                    fstate["i"] += 1
                    if item[0] == "stage_b":
                        wb_refs[item[1]] = {"wb": emit_wqk_stage(item[1])}
                        continue
                    if item[0] == "stage_wv":
                        emit_wv_chunk(item[1])
                        continue
                    if item[0] == "stage_wo":
                        ko = item[1]
                        ws = wos_pool.tile([P, 1, D], f32, tag="wos", name="wos")
                        nc.sync.dma_start(ws[:], w_out_kp[:, ko : ko + 1, :])
                        nc.vector.tensor_copy(wout[:, ko : ko + 1, :], ws[:])
                        continue
                    if item[0] == "bgroup":
                        fstate["group"] = make_b_group(
                            item[1], item[2], wb_refs[item[1]]
                        )
                    else:
                        fstate["group"] = make_c_group(item[1], item[2])
                    fstate["mmi"] = 0
                    fstate["item"] = item
                g = fstate["group"]
                g["mms"][fstate["mmi"]]()
                fstate["mmi"] += 1
                mms += 1
                cyc += 530
                if fstate["mmi"] == DT:
                    g["drain"]()
                    if fstate["item"][0] == "bgroup":
                        nt = fstate["item"][1]
                        b_done[nt] = b_done.get(nt, 0) + 1
                    fstate["group"] = None
            return cyc

        def force_b(nt):
            cyc = 0
            while b_done.get(nt, 0) < 2 and not filler_exhausted():
                cyc += emit_filler_unit(max_mms=8)
            return cyc

        def force_c(ncx):
            cyc = 0
            while not c_done(ncx) and not filler_exhausted():
                cyc += emit_filler_unit(max_mms=8)
            return cyc

        # ---- scores / PV / norm ----------------------------------------
        exs = {}
        pv_state = {"w": 0, "sk": 0, "tiles": None}

        def emit_score_step(w, p, sc, sk):
            sps = sps_pool.tile([P, S], f32, tag="sps", name="sps")
            for hh in range(2):
                base = hh * DK
                nc.tensor.matmul(
                    sps[:, hh * 512 : (hh + 1) * 512],
                    qkT[base : base + DK, PAIRS + p, sk * P : (sk + 1) * P],
                    qkT[base : base + DK, p, sc * 512 : (sc + 1) * 512],
                    start=True,
                    stop=True,
                )
            ex = ex_pool.tile([P, S], bf16, tag="ex", name="ex")
            nc.scalar.activation(ex[:], sps[:], EXP, scale=1.0 / np.sqrt(DK))
            exs[(w, sk)] = ex

        def norm_window(w, pvt):
            p2, sc2 = divmod(w, 2)
            pvs = []
            for hh in range(2):
                pv_sb = pvs_pool.tile([P, 512], f32, tag="pvs", name="pvsb")
                nc.vector.tensor_copy(pv_sb[0 : DK + 1, :], pvt[hh][0 : DK + 1, :])
                pvs.append(pv_sb)
            dnc = dnc_pool.tile([8, P], f32, tag="dnc", name="dnc")
            for hh in range(2):
                nc.sync.dma_start(
                    dnc[4 * hh : 4 * hh + 4, :], pvs[hh][DK : DK + 1, :]
                )
            rc = rc_pool.tile([8, P], bf16, tag="rc", name="rc")
            with nc.allow_low_precision(reason="1/denom fits bf16 (0.4% rel)"):
                nc.vector.reciprocal(rc[:], dnc[:])
            for hh in range(2):
                rr = rr_pool.tile([1, 512], bf16, tag="rr", name="rr")
                nc.sync.dma_start(rr[:], rc[4 * hh : 4 * hh + 4, :])
                bcc = bc_pool.tile([P, 512], bf16, tag="bc", name="bcc")
                nc.gpsimd.partition_broadcast(bcc[:], rr[:])
                nc.vector.tensor_tensor(
                    attnT[DK * hh : DK * hh + DK, p2, sc2 * 512 : (sc2 + 1) * 512],
                    pvs[hh][0:DK, :],
                    bcc[0:DK, :],
                    MULT,
                )

        def pv_ready():
            w = pv_state["w"]
            if w >= NW:
                return False
            if not c_done((w // 2) // 4):
                return False
            return (w, pv_state["sk"]) in exs

        def emit_pv_step():
            w = pv_state["w"]
            sk = pv_state["sk"]
            p2 = w // 2
            if sk == 0:
                pv_state["tiles"] = [
                    pv_ps.tile([P, 512], f32, tag="pv", name=f"pvt{hh}")
                    for hh in range(2)
                ]
            ex = exs[(w, sk)]
            for hh in range(2):
                nc.tensor.matmul(
                    pv_state["tiles"][hh][0 : DK + 1, :],
                    v_aug[:, sk, 2 * p2 + hh, :],
                    ex[:, hh * 512 : (hh + 1) * 512],
                    start=(sk == 0),
                    stop=(sk == ST - 1),
                )
            del exs[(w, sk)]
            pv_state["sk"] += 1
            if pv_state["sk"] == ST:
                norm_window(w, pv_state["tiles"])
                pv_state["w"] += 1
                pv_state["sk"] = 0
                pv_state["tiles"] = None

        # ------------------------------------------------ head, part 2
        for nt in (0, 8):
            g = make_b_group(nt, 0, wb_refs[nt])
            for m in g["mms"]:
                m()
            g["drain"]()
            b_done[nt] += 1
        for sk in range(4):
            emit_score_step(0, 0, 0, sk)
        for si in range(4, ST):
            emit_a_step(si)
        for nt in (8, 0):
            g = make_b_group(nt, 1, wb_refs[nt])
            for m in g["mms"]:
                m()
            g["drain"]()
            b_done[nt] += 1
        for sk in range(4, ST):
            emit_score_step(0, 0, 0, sk)
            emit_filler_unit(max_mms=2)
        rowp.release()
        xbp.release()
        xstage.release()

        # W_out: resident bf16, staged chunk-wise during the windows so the
        # output projection can start the moment the last attnT lands
        wout_pool = tc.alloc_tile_pool(name="woutp", bufs=1)
        wout = wout_pool.tile([P, DT, D], bf16)
        wos_pool = tc.alloc_tile_pool(name="wosp", bufs=1)
        brow2 = wos_pool.tile([1, D], f32, tag="br", name="borow")
        nc.sync.dma_start(brow2[:], b_out[None, :])
        nc.gpsimd.partition_broadcast(bout_bc[:], brow2[:])

        # ------------------------------------------------ windows 1..15
        ACT_CYC = 2750
        budget = 0.0
        for w in range(1, NW):
            p, sc = divmod(w, 2)
            # safety net: this window's own projections (normally done)
            budget -= force_b(p)
            budget -= force_b(PAIRS + p)
            if w == 5:
                budget -= force_c(0)
            elif w == 10:
                budget -= force_c(1)
            for sk in range(ST):
                emit_score_step(w, p, sc, sk)
                budget += ACT_CYC - 560
                npv = 0
                while pv_ready() and npv < 4:
                    emit_pv_step()
                    budget -= 1060
                    npv += 1
                # inject filler in whole-group runs when possible: uniform
                # back-to-back matmul runs keep LDWEIGHTS pipelined
                while budget > 600 and not filler_exhausted():
                    got = emit_filler_unit(
                        max_mms=8 if budget > 4400 else 2
                    )
                    if got == 0:
                        break
                    budget -= got
                budget = min(budget, 8000.0)

        # ------------------------------------------------ tail
        while not filler_exhausted():
            emit_filler_unit(max_mms=8)
        while pv_state["w"] < NW:
            emit_pv_step()

        for pool in (wv_stage, ex_pool, wv_pool, wqk_bfp, wqk_stage, xT_pool):
            pool.release()

        with tc.tile_pool(name="ypool", bufs=3, side="right") as ypool:
            for st in range(ST):
                ps = [
                    bce_ps.tile([P, 512], f32, tag="bce", name=f"pse{ncx}")
                    for ncx in range(2)
                ]
                for dkk in range(DT):
                    for ncx in range(2):
                        nc.tensor.matmul(
                            ps[ncx][:],
                            attnT[:, dkk, st * P : (st + 1) * P],
                            wout[:, dkk, ncx * 512 : (ncx + 1) * 512],
                            start=(dkk == 0),
                            stop=(dkk == DT - 1),
                        )
                for ncx in range(2):
                    y = ypool.tile([P, 512], f32, tag="y", name="y")
                    nc.vector.tensor_tensor(
                        y[:], ps[ncx][:], bout_bc[:, ncx * 512 : (ncx + 1) * 512],
                        ADD,
                    )
                    nc.sync.dma_start(
                        out[st * P : (st + 1) * P, ncx * 512 : (ncx + 1) * 512],
                        y[:],
                    )

        for pool in (
            pv_ps, sps_pool, bce_ps,
            wos_pool, wout_pool,
            bc_pool, rr_pool, rc_pool, dnc_pool, pvs_pool,
            attnT_pool, vaug_pool, qkT_pool, const,
        ):
            pool.release()

    nc.finalize()
    return nc


_NC_CACHE = {}


def get_nc():
    if "nc" not in _NC_CACHE:
        _NC_CACHE["nc"] = build_nc()
    return _NC_CACHE["nc"]


def kernel(X, W_in, b_in, W_out, b_out):
    X = np.ascontiguousarray(np.asarray(X, dtype=np.float32))
    W_in = np.ascontiguousarray(np.asarray(W_in, dtype=np.float32))
    b_in = np.ascontiguousarray(np.asarray(b_in, dtype=np.float32))
    W_out = np.ascontiguousarray(np.asarray(W_out, dtype=np.float32))
    b_out = np.ascontiguousarray(np.asarray(b_out, dtype=np.float32))

    nc = get_nc()
    in_maps = [
        {"X": X[i], "W_in": W_in, "b_in": b_in, "W_out": W_out, "b_out": b_out}
        for i in range(B)
    ]
    res = run_bass_kernel_spmd(nc, in_maps, core_ids=list(range(B)))
    return np.stack([res.results[i]["out"] for i in range(B)], axis=0)


# revision 84
# speedup vs baseline: 1.0282x; 1.0282x over previous
"""Multi-head attention (B=8, S=1024, D=1024, H=16) on 8 TRN2 NeuronCores.

Sharding: pure data parallel - batch element b on core b. Weights broadcast.

Single-core schedule: a 16-window (head-pair x seq-half) software pipeline
keeping ScalarE (exp, ~147us) and TensorE (~200us of matmul streaming)
concurrent from ~15us:

  head:    X^T via PE transposes (ScalarE drains them so DVE stays free
           for casts); Q/K projection for pair 0; window-0 scores fire as
           soon as s-tiles 0-3 land, starting the exp spine at ~20us.
  windows: per (pair, sc) window the 8 score matmuls + one wide exp per
           k-tile set the ACT pace; PV steps for the trailing window and
           filler (remaining projections, V projection, W_out staging)
           absorb TensorE idle. Hard deadlines force projection emission
           before the window that reads it.
  PV:      ones-augmented V (M=65) accumulates unnormalized out^T plus the
           softmax denominator; denominators are reshaped [1,512]->[4,128]
           by SBUF-SBUF DMA so one [8,128] DVE reciprocal per window
           replaces 2 single-partition 3.3us reciprocals.
  tail:    remaining PV windows + normalize, then Y = attn_out @ W_out.
"""

import sys

sys.path.insert(0, "/opt/trn_rl_repo")

import numpy as np

import concourse.bacc as bacc
import concourse.mybir as mybir
from concourse.bass_utils import run_bass_kernel_spmd
from concourse.masks import make_identity
from concourse.tile import TileContext

B = 8
S = 1024
D = 1024
H = 16
DK = D // H  # 64
P = 128
ST = S // P   # 8 s-tiles
DT = D // P   # 8 d-tiles
NTQK = 2 * D // P  # 16 n-tiles for the Q|K part
PAIRS = H // 2     # 8 head pairs
NW = 2 * PAIRS     # 16 windows (pair, sc)

f32 = mybir.dt.float32
bf16 = mybir.dt.bfloat16
EXP = mybir.ActivationFunctionType.Exp
MULT = mybir.AluOpType.mult
ADD = mybir.AluOpType.add


def build_nc():
    nc = bacc.Bacc()
    X = nc.dram_tensor("X", [S, D], f32, kind="ExternalInput")
    W_in = nc.dram_tensor("W_in", [D, 3 * D], f32, kind="ExternalInput")
    b_in = nc.dram_tensor("b_in", [3 * D], f32, kind="ExternalInput")
    W_out = nc.dram_tensor("W_out", [D, D], f32, kind="ExternalInput")
    b_out = nc.dram_tensor("b_out", [D], f32, kind="ExternalInput")
    out = nc.dram_tensor("out", [S, D], f32, kind="ExternalOutput")

    w_in_kp = W_in.rearrange("(ko p) n -> p ko n", p=P)  # [128, 8, 3072]
    w_out_kp = W_out.rearrange("(ko p) n -> p ko n", p=P)  # [128, 8, 1024]

    with TileContext(nc) as tc:
        # ------------------------------------------------ constants
        const = tc.alloc_tile_pool(name="const", bufs=1)
        bqk = const.tile([P, NTQK], f32)
        bv_bc = const.tile([P, D], f32)
        bout_bc = const.tile([P, D], f32)
        ones4 = const.tile([P, ST, H, 1], f32)
        nc.vector.memset(ones4[:], 1.0)

        # ------------------------------------------------ resident (left)
        qkT_pool = tc.alloc_tile_pool(name="qkT", bufs=1)
        qkT = qkT_pool.tile([P, NTQK, S], bf16)  # 32 KB/p
        vaug_pool = tc.alloc_tile_pool(name="vaug", bufs=1)
        v_aug = vaug_pool.tile([P, ST, H, DK + 1], bf16)  # 16.6 KB/p
        nc.vector.tensor_copy(v_aug[:, :, :, DK : DK + 1], ones4[:])
        attnT_pool = tc.alloc_tile_pool(name="attnT", bufs=1)
        attnT = attnT_pool.tile([P, PAIRS, S], bf16)  # 16 KB/p

        pvs_pool = tc.alloc_tile_pool(name="pvs", bufs=3)   # 6 KB/p
        dnc_pool = tc.alloc_tile_pool(name="dnc", bufs=2)
        rc_pool = tc.alloc_tile_pool(name="rcp", bufs=2)
        rr_pool = tc.alloc_tile_pool(name="rrp", bufs=2)
        bc_pool = tc.alloc_tile_pool(name="bcp", bufs=2)

        # ------------------------------------------------ transient (right)
        xT_pool = tc.alloc_tile_pool(name="xTp", bufs=1, side="right")
        xT = xT_pool.tile([P, DT, S], bf16)  # 16 KB/p
        wqk_stage = tc.alloc_tile_pool(name="wqks", bufs=2, side="right")
        wqk_bfp = tc.alloc_tile_pool(name="wqkb", bufs=2, side="right")
        wv_pool = tc.alloc_tile_pool(name="wvb", bufs=1, side="right")
        wv_bf = wv_pool.tile([P, DT, D], bf16)  # 16 KB/p
        ex_pool = tc.alloc_tile_pool(name="exp", bufs=22, side="right")  # 44
        wv_stage = tc.alloc_tile_pool(name="wvs", bufs=1, side="right")

        def emit_wqk_stage(nt):
            ws = wqk_stage.tile([P, DT, P], f32, tag="ws", name="ws")
            nc.sync.dma_start(ws[:], w_in_kp[:, :, nt * P : (nt + 1) * P])
            wb = wqk_bfp.tile([P, DT, P], bf16, tag="wb", name="wb")
            nc.vector.tensor_copy(wb[:], ws[:])
            return wb

        wb_refs = {}

        # ------------------------------------------------ PSUM pools
        # bce (filler/E) 2 banks, sps 4 banks, pv 2 banks = 8
        bce_ps = tc.alloc_tile_pool(name="bceps", bufs=2, space="PSUM")
        sps_pool = tc.alloc_tile_pool(name="spsp", bufs=2, space="PSUM")
        pv_ps = tc.alloc_tile_pool(name="pvps", bufs=2, space="PSUM")

        # phase A staging
        xstage = tc.alloc_tile_pool(name="xstage", bufs=2, side="right")
        xbp = tc.alloc_tile_pool(name="xbp", bufs=2, side="right")
        rowp = tc.alloc_tile_pool(name="rowp", bufs=1, side="right")

        identity = const.tile([P, P], bf16)
        make_identity(nc, identity[:])

        def emit_a_step(si, eng=None):
            x_tile = xstage.tile([P, D], f32, tag="x", name="xt")
            (eng or nc.sync).dma_start(x_tile[:], X[si * P : (si + 1) * P, :])
            xb = xbp.tile([P, D], bf16, tag="xb", name="xb")
            nc.vector.tensor_copy(xb[:], x_tile[:])
            for half in range(2):
                tp4 = bce_ps.tile([P, 4, P], f32, tag="bce", name="tp4")
                for dj in range(4):
                    dj_abs = half * 4 + dj
                    nc.tensor.matmul(
                        tp4[:, dj, :],
                        xb[:, dj_abs * P : (dj_abs + 1) * P],
                        identity[:],
                        start=True,
                        stop=True,
                    )
                # ScalarE is idle during the head; keep DVE free
                nc.scalar.copy(
                    xT[:, half * 4 : (half + 1) * 4, si * P : (si + 1) * P],
                    tp4[:],
                )

        for si in range(4):
            emit_a_step(si)
            if si == 1:
                wb_refs[0] = {"wb": emit_wqk_stage(0)}
            elif si == 2:
                wb_refs[8] = {"wb": emit_wqk_stage(8)}
            elif si == 3:
                brow = rowp.tile([1, D], f32, tag="r", name="bvrow")
                nc.sync.dma_start(brow[:], b_in[None, 2 * D : 3 * D])
                nc.gpsimd.partition_broadcast(bv_bc[:], brow[:])
                nc.sync.dma_start(
                    bqk[:], b_in[0 : 2 * D].rearrange("(o p) -> p o", p=P)
                )

        # ------------------------------------------------ helpers
        def emit_wv_chunk(ci):
            # chunk ci: dk pair (2*ci, 2*ci+1), all 1024 V columns
            wvs = wv_stage.tile([P, 2, D], f32, tag="wvs", name="wvs")
            nc.sync.dma_start(
                wvs[:], w_in_kp[:, 2 * ci : 2 * ci + 2, 2 * D : 3 * D]
            )
            nc.vector.tensor_copy(wv_bf[:, 2 * ci : 2 * ci + 2, :], wvs[:])

        def make_b_group(nt, sc, wb_ref):
            ps_ref = {}

            def mm(dk):
                if dk == 0:
                    ps_ref["ps"] = bce_ps.tile([P, 512], f32, tag="bce", name="psb")
                nc.tensor.matmul(
                    ps_ref["ps"][:],
                    wb_ref["wb"][:, dk, :],
                    xT[:, dk, sc * 512 : (sc + 1) * 512],
                    start=(dk == 0),
                    stop=(dk == DT - 1),
                )

            def drain():
                nc.vector.tensor_scalar_add(
                    qkT[:, nt, sc * 512 : (sc + 1) * 512],
                    ps_ref["ps"][:],
                    bqk[:, nt : nt + 1],
                )

            return {"mms": [lambda dk=dk: mm(dk) for dk in range(DT)],
                    "drain": drain}

        def make_c_group(st, ncx):
            ps_ref = {}

            def mm(dk):
                if dk == 0:
                    ps_ref["ps"] = bce_ps.tile([P, 512], f32, tag="bce", name="psc")
                nc.tensor.matmul(
                    ps_ref["ps"][:],
                    xT[:, dk, st * P : (st + 1) * P],
                    wv_bf[:, dk, ncx * 512 : (ncx + 1) * 512],
                    start=(dk == 0),
                    stop=(dk == DT - 1),
                )

            def drain():
                nc.vector.tensor_tensor(
                    v_aug[:, st, ncx * 8 : (ncx + 1) * 8, 0:DK],
                    ps_ref["ps"][:].rearrange("p (h d) -> p h d", d=DK),
                    bv_bc[:, ncx * 512 : (ncx + 1) * 512].rearrange(
                        "p (h d) -> p h d", d=DK
                    ),
                    ADD,
                )
                c_emitted[ncx] += 1

            return {"mms": [lambda dk=dk: mm(dk) for dk in range(DT)],
                    "drain": drain}

        c_emitted = {0: 0, 1: 0}

        def c_done(ncx):
            return c_emitted[ncx] >= ST

        # ---- filler worklist (deadline ordered) ------------------------
        filler_plan = []

        def plan_b(nt):
            filler_plan.append(("stage_b", nt))
            filler_plan.append(("bgroup", nt, 0))
            filler_plan.append(("bgroup", nt, 1))

        # weave: each nt's weight stage (DMA+cast) lands one group-pair
        # ahead of its matmuls so the cast never blocks the PE
        b_order = [1, 9, 2, 10, 3, 11, 4, 12, 5, 13, 6, 14, 7, 15]
        other = (
            [("stage_wv", ci) for ci in range(4)]
            + [("cgroup", st, 0) for st in range(ST)]
            + [("cgroup", st, 1) for st in range(ST)]
            + [("stage_wo", ko) for ko in range(DT)]
        )
        oi = [0]

        def take_other(n):
            got = other[oi[0] : oi[0] + n]
            oi[0] += len(got)
            return got

        filler_plan.append(("stage_b", b_order[0]))
        filler_plan.append(("stage_b", b_order[1]))
        filler_plan.extend(take_other(2))  # wv chunks 0,1 early
        for k, nt in enumerate(b_order):
            filler_plan.append(("bgroup", nt, 0))
            if k + 2 < len(b_order):
                filler_plan.append(("stage_b", b_order[k + 2]))
            filler_plan.append(("bgroup", nt, 1))
            filler_plan.extend(take_other(2))
        filler_plan.extend(other[oi[0] :])

        fstate = {"i": 0, "group": None, "mmi": 0, "item": None}
        b_done = {0: 0, 8: 0}

        def filler_exhausted():
            return fstate["group"] is None and fstate["i"] >= len(filler_plan)

        def emit_filler_unit(max_mms=2):
            cyc = 0
            mms = 0
            while mms < max_mms:
                if fstate["group"] is None:
                    if fstate["i"] >= len(filler_plan):
                        return cyc
                    item = filler_plan[fstate["i"]]
                    fstate["i"] += 1
                    if item[0] == "stage_b":
                        wb_refs[item[1]] = {"wb": emit_wqk_stage(item[1])}
                        continue
                    if item[0] == "stage_wv":
                        emit_wv_chunk(item[1])
                        continue
                    if item[0] == "stage_wo":
                        ko = item[1]
                        ws = wos_pool.tile([P, 1, D], f32, tag="wos", name="wos")
                        nc.sync.dma_start(ws[:], w_out_kp[:, ko : ko + 1, :])
                        nc.vector.tensor_copy(wout[:, ko : ko + 1, :], ws[:])
                        continue
                    if item[0] == "bgroup":
                        fstate["group"] = make_b_group(
                            item[1], item[2], wb_refs[item[1]]
                        )
                    else:
                        fstate["group"] = make_c_group(item[1], item[2])
                    fstate["mmi"] = 0
                    fstate["item"] = item
                g = fstate["group"]
                g["mms"][fstate["mmi"]]()
                fstate["mmi"] += 1
                mms += 1
                cyc += 530
                if fstate["mmi"] == DT:
                    g["drain"]()
                    if fstate["item"][0] == "bgroup":
                        nt = fstate["item"][1]
                        b_done[nt] = b_done.get(nt, 0) + 1
                    fstate["group"] = None
            return cyc

        def force_b(nt):
            cyc = 0
            while b_done.get(nt, 0) < 2 and not filler_exhausted():
                cyc += emit_filler_unit(max_mms=8)
            return cyc

        def force_c(ncx):
            cyc = 0
            while not c_done(ncx) and not filler_exhausted():
                cyc += emit_filler_unit(max_mms=8)
            return cyc

        # ---- scores / PV / norm ----------------------------------------
        exs = {}
        pv_state = {"w": 0, "sk": 0, "tiles": None}

        def emit_score_step(w, p, sc, sk):
            sps = sps_pool.tile([P, S], f32, tag="sps", name="sps")
            for hh in range(2):
                base = hh * DK
                nc.tensor.matmul(
                    sps[:, hh * 512 : (hh + 1) * 512],
                    qkT[base : base + DK, PAIRS + p, sk * P : (sk + 1) * P],
                    qkT[base : base + DK, p, sc * 512 : (sc + 1) * 512],
                    start=True,
                    stop=True,
                )
            ex = ex_pool.tile([P, S], bf16, tag="ex", name="ex")
            nc.scalar.activation(ex[:], sps[:], EXP, scale=1.0 / np.sqrt(DK))
            exs[(w, sk)] = ex

        def norm_window(w, pvt):
            p2, sc2 = divmod(w, 2)
            pvs = []
            for hh in range(2):
                pv_sb = pvs_pool.tile([P, 512], f32, tag="pvs", name="pvsb")
                nc.vector.tensor_copy(pv_sb[0 : DK + 1, :], pvt[hh][0 : DK + 1, :])
                pvs.append(pv_sb)
            dnc = dnc_pool.tile([8, P], f32, tag="dnc", name="dnc")
            for hh in range(2):
                nc.sync.dma_start(
                    dnc[4 * hh : 4 * hh + 4, :], pvs[hh][DK : DK + 1, :]
                )
            rc = rc_pool.tile([8, P], bf16, tag="rc", name="rc")
            with nc.allow_low_precision(reason="1/denom fits bf16 (0.4% rel)"):
                nc.vector.reciprocal(rc[:], dnc[:])
            for hh in range(2):
                rr = rr_pool.tile([1, 512], bf16, tag="rr", name="rr")
                nc.sync.dma_start(rr[:], rc[4 * hh : 4 * hh + 4, :])
                bcc = bc_pool.tile([P, 512], bf16, tag="bc", name="bcc")
                nc.gpsimd.partition_broadcast(bcc[:], rr[:])
                nc.vector.tensor_tensor(
                    attnT[DK * hh : DK * hh + DK, p2, sc2 * 512 : (sc2 + 1) * 512],
                    pvs[hh][0:DK, :],
                    bcc[0:DK, :],
                    MULT,
                )

        def pv_ready():
            w = pv_state["w"]
            if w >= NW:
                return False
            if not c_done((w // 2) // 4):
                return False
            return (w, pv_state["sk"]) in exs

        def emit_pv_step():
            w = pv_state["w"]
            sk = pv_state["sk"]
            p2 = w // 2
            if sk == 0:
                pv_state["tiles"] = [
                    pv_ps.tile([P, 512], f32, tag="pv", name=f"pvt{hh}")
                    for hh in range(2)
                ]
            ex = exs[(w, sk)]
            for hh in range(2):
                nc.tensor.matmul(
                    pv_state["tiles"][hh][0 : DK + 1, :],
                    v_aug[:, sk, 2 * p2 + hh, :],
                    ex[:, hh * 512 : (hh + 1) * 512],
                    start=(sk == 0),
                    stop=(sk == ST - 1),
                )
            del exs[(w, sk)]
            pv_state["sk"] += 1
            if pv_state["sk"] == ST:
                norm_window(w, pv_state["tiles"])
                pv_state["w"] += 1
                pv_state["sk"] = 0
                pv_state["tiles"] = None

        # ------------------------------------------------ head, part 2
        for nt in (0, 8):
            g = make_b_group(nt, 0, wb_refs[nt])
            for m in g["mms"]:
                m()
            g["drain"]()
            b_done[nt] += 1
        for sk in range(4):
            emit_score_step(0, 0, 0, sk)
        for si in range(4, ST):
            emit_a_step(si)
        for nt in (8, 0):
            g = make_b_group(nt, 1, wb_refs[nt])
            for m in g["mms"]:
                m()
            g["drain"]()
            b_done[nt] += 1
        for sk in range(4, ST):
            emit_score_step(0, 0, 0, sk)
            emit_filler_unit(max_mms=2)
        rowp.release()
        xbp.release()
        xstage.release()

        # W_out: resident bf16, staged chunk-wise during the windows so the
        # output projection can start the moment the last attnT lands
        wout_pool = tc.alloc_tile_pool(name="woutp", bufs=1)
        wout = wout_pool.tile([P, DT, D], bf16)
        wos_pool = tc.alloc_tile_pool(name="wosp", bufs=1)
        brow2 = wos_pool.tile([1, D], f32, tag="br", name="borow")
        nc.sync.dma_start(brow2[:], b_out[None, :])
        nc.gpsimd.partition_broadcast(bout_bc[:], brow2[:])

        # ------------------------------------------------ windows 1..15
        ACT_CYC = 2750
        budget = 0.0
        for w in range(1, NW):
            p, sc = divmod(w, 2)
            # safety net: this window's own projections (normally done)
            budget -= force_b(p)
            budget -= force_b(PAIRS + p)
            if w == 5:
                budget -= force_c(0)
            elif w == 10:
                budget -= force_c(1)
            for sk in range(ST):
                emit_score_step(w, p, sc, sk)
                budget += ACT_CYC - 560
                npv = 0
                while pv_ready() and npv < 4:
                    emit_pv_step()
                    budget -= 1060
                    npv += 1
                # inject filler in whole-group runs when possible: uniform
                # back-to-back matmul runs keep LDWEIGHTS pipelined
                while budget > 600 and not filler_exhausted():
                    got = emit_filler_unit(
                        max_mms=8 if budget > 4400 else 2
                    )
                    if got == 0:
                        break
                    budget -= got
                budget = min(budget, 8000.0)

        # ------------------------------------------------ tail
        while not filler_exhausted():
            emit_filler_unit(max_mms=8)
        while pv_state["w"] < NW:
            emit_pv_step()

        for pool in (wv_stage, ex_pool, wv_pool, wqk_bfp, wqk_stage, xT_pool):
            pool.release()

        with tc.tile_pool(name="ypool", bufs=3, side="right") as ypool:
            for st in range(ST):
                ps = [
                    bce_ps.tile([P, 512], f32, tag="bce", name=f"pse{ncx}")
                    for ncx in range(2)
                ]
                for dkk in range(DT):
                    for ncx in range(2):
                        nc.tensor.matmul(
                            ps[ncx][:],
                            attnT[:, dkk, st * P : (st + 1) * P],
                            wout[:, dkk, ncx * 512 : (ncx + 1) * 512],
                            start=(dkk == 0),
                            stop=(dkk == DT - 1),
                        )
                for ncx in range(2):
                    y = ypool.tile([P, 512], f32, tag="y", name="y")
                    nc.vector.tensor_tensor(
                        y[:], ps[ncx][:], bout_bc[:, ncx * 512 : (ncx + 1) * 512],
                        ADD,
                    )
                    nc.sync.dma_start(
                        out[st * P : (st + 1) * P, ncx * 512 : (ncx + 1) * 512],
                        y[:],
                    )

        for pool in (
            pv_ps, sps_pool, bce_ps,
            wos_pool, wout_pool,
            bc_pool, rr_pool, rc_pool, dnc_pool, pvs_pool,
            attnT_pool, vaug_pool, qkT_pool, const,
        ):
            pool.release()

    nc.finalize()
    return nc


_NC_CACHE = {}


def get_nc():
    if "nc" not in _NC_CACHE:
        _NC_CACHE["nc"] = build_nc()
    return _NC_CACHE["nc"]


def kernel(X, W_in, b_in, W_out, b_out):
    X = np.ascontiguousarray(np.asarray(X, dtype=np.float32))
    W_in = np.ascontiguousarray(np.asarray(W_in, dtype=np.float32))
    b_in = np.ascontiguousarray(np.asarray(b_in, dtype=np.float32))
    W_out = np.ascontiguousarray(np.asarray(W_out, dtype=np.float32))
    b_out = np.ascontiguousarray(np.asarray(b_out, dtype=np.float32))

    nc = get_nc()
    in_maps = [
        {"X": X[i], "W_in": W_in, "b_in": b_in, "W_out": W_out, "b_out": b_out}
        for i in range(B)
    ]
    res = run_bass_kernel_spmd(nc, in_maps, core_ids=list(range(B)))
    return np.stack([res.results[i]["out"] for i in range(B)], axis=0)
